# revision 1
# baseline (speedup 1.0000x reference)
"""PointGNN Trainium2 kernel (nn_PointGNN_11931419149118).

Algebraic collapse of the reference: the edge-MLP input is
concat(zeros(3), adj ? state[j] : 0), so for adjacent (i,j) the edge
feature E[j] = MLP_f([0, state[j]]) depends only on j. Since MLP_f ends
in a ReLU and e is re-masked by adj before the max over j,
    agg[i, c] = max_j adj[i, j] * E'[j, c]        (E' = pre-relu edge MLP)
where the zeros contributed by non-neighbors supply the final ReLU for
free (max(0, .) == relu, and every point has non-neighbors). This
avoids materializing the reference's (N, M, M, 128) tensors entirely.

Mapping: the masked max runs on the vector engine in fp16 as one
group-batched mult (adjacency broadcast across channel groups via
0-stride APs) + a pairwise-max tree ending in a narrow reduce; E' rows
are broadcast across partitions by tensor-engine "selector" matmuls
(lhsT = e_c x ones, a zero-stride AP view of an identity tile); the
scalar engine converts PSUM results to fp16 SBUF. MLP weights live in
one packed fp16 blob (single DMA, single-pass matmuls); biases in a
small fp32 blob. The state residual is accumulated into a PSUM by
fp32 identity matmuls, keeping MLP_g entirely off the vector engine.

Sharding (8 cores): cores [4q, 4q+4) own frames {2q, 2q+1}, each core
taking a 32-channel slice of the 128 edge channels for BOTH frames.
The two frames are software-pipelined: stream s's agg AllGather, MLP_g,
and the next timestep's edge MLP + broadcasts all execute under the
other stream's ~40us of masked-max vector work.
"""

import sys
import types

sys.path.insert(0, "/opt/trn_rl_repo")

import numpy as np
from contextlib import ExitStack

import concourse.bass as bass
import concourse.mybir as mybir
import concourse.tile as tile
from concourse import bacc
from concourse.bass_utils import run_bass_kernel_spmd
from concourse.masks import make_identity

F32 = mybir.dt.float32
F16 = mybir.dt.float16
AF = mybir.ActivationFunctionType
ALU = mybir.AluOpType
AX = mybir.AxisListType

N_FRAMES = 4
M = 384          # points per frame
P = 128          # partitions
NB = M // P      # 3 destination blocks
T = 3            # timesteps
C = 128          # edge channels
NS = 2           # frame streams per core
CH = C // 4      # channels per core (quarter)
G = 8            # channel group size for batched DVE ops
NG = CH // G     # groups per core per stream
R = 0.05         # squared-distance threshold
N_CORES = 8
REPLICA_GROUPS = [[0, 1, 2, 3], [4, 5, 6, 7]]

# packed fp16 weight blob layout: per t, (rows, cols) per weight
_W16 = [("fW1s", 3, 64), ("fW2", 64, C), ("fW3c", C, CH),
        ("gW1", C, 64), ("gW1h0", C // 2, 64), ("gW1h1", C // 2, 64),
        ("gW2", 64, 32), ("gW3", 32, 3)]
_W16_COLS = sum(c for _, _, c in _W16)           # per timestep
_B32 = [("fb1", 64), ("fb2", C), ("fb3c", CH), ("gb1", 64),
        ("gb2", 32), ("gb3", 3)]


def _w16_off(name, t):
    off = t * _W16_COLS
    for n, _, c in _W16:
        if n == name:
            return off
        off += c
    raise KeyError(name)


def _b32_off(name, t):
    off = t * len(_B32)
    for i, (n, _) in enumerate(_B32):
        if n == name:
            return off + i
    raise KeyError(name)


def _register_ntff_hook():
    """Register the axon NTFF profile hook the image's antenv lacks."""
    try:
        import antenv
        if "antenv.axon_hooks" in sys.modules:
            return
        mod = types.ModuleType("antenv.axon_hooks")
        _hook = [None]
        mod.set_axon_ntff_profile_hook = lambda h: _hook.__setitem__(0, h)
        mod.get_axon_ntff_profile_hook = lambda: _hook[0]
        sys.modules["antenv.axon_hooks"] = mod
        antenv.axon_hooks = mod
        from trn_agent_boot.trn_boot import _ntff_profile_via_ctypes
        mod.set_axon_ntff_profile_hook(
            _ntff_profile_via_ctypes("/opt/axon/libaxon_pjrt.so")
        )
    except Exception:
        pass


def build(ctx, tc):
    nc = tc.nc

    x_in = nc.declare_dram_parameter("x", [P, NS, NB, 3], F32,
                                     isOutput=False)
    xT_in = nc.declare_dram_parameter("xT", [3, NS, M], F32, isOutput=False)
    xT16_in = nc.declare_dram_parameter("xT16", [3, NS, M], F16,
                                        isOutput=False)
    wb16_in = nc.declare_dram_parameter("wb16", [P, T * _W16_COLS], F16,
                                        isOutput=False)
    wb32_in = nc.declare_dram_parameter("wb32", [P, T * len(_B32)], F32,
                                        isOutput=False)
    out_ext = nc.declare_dram_parameter("out", [NS, 3, M], F32, isOutput=True)

    agg_out = [[nc.dram_tensor(f"agg_out_t{t}s{s}", [CH, M], F16)
                for s in range(NS)] for t in range(T)]
    agg_full = [[nc.dram_tensor(f"agg_full_t{t}s{s}", [4, CH, M], F16)
                 for s in range(NS)] for t in range(T)]

    consts = ctx.enter_context(tc.tile_pool(name="consts", bufs=1))
    scratch_pool = ctx.enter_context(tc.tile_pool(name="scratch", bufs=3))
    work = ctx.enter_context(tc.tile_pool(name="work", bufs=2))
    ebc_pool = ctx.enter_context(tc.tile_pool(name="ebc", bufs=4))
    mg_pool = ctx.enter_context(tc.tile_pool(name="mg", bufs=2))
    psum = ctx.enter_context(
        tc.tile_pool(name="psum", bufs=2, space=bass.MemorySpace.PSUM)
    )
    psum_bc = ctx.enter_context(
        tc.tile_pool(name="psum_bc", bufs=2, space=bass.MemorySpace.PSUM)
    )
    psum_g = ctx.enter_context(
        tc.tile_pool(name="psum_g", bufs=1, space=bass.MemorySpace.PSUM)
    )

    # ---- x loads first (adjacency is on the DVE critical path) ----
    xall = consts.tile([P, NS, NB, 3], F32, tag="xall", name="xall")
    nc.sync.dma_start(out=xall, in_=x_in[:])
    xs = [xall[:, s] for s in range(NS)]
    xTf = consts.tile([3, NS, M], F32, tag="xTf", name="xTf")
    nc.sync.dma_start(out=xTf, in_=xT_in[:])
    xT16f = consts.tile([3, NS, M], F16, tag="xT16f", name="xT16f")
    nc.sync.dma_start(out=xT16f, in_=xT16_in[:])

    # ---- packed weights: one DMA each ----
    wb16 = consts.tile([P, T * _W16_COLS], F16, tag="wb16", name="wb16")
    nc.sync.dma_start(out=wb16, in_=wb16_in[:])
    wb32 = consts.tile([P, T * len(_B32)], F32, tag="wb32", name="wb32")
    nc.sync.dma_start(out=wb32, in_=wb32_in[:])

    def wt(name, t):
        for n, r, c in _W16:
            if n == name:
                o = _w16_off(name, t)
                return wb16[:r, o:o + c]
        for n, r in _B32:
            if n == name:
                return wb32[:r, _b32_off(name, t):_b32_off(name, t) + 1]
        raise KeyError(name)

    identity = consts.tile([P, P], F32, tag="identity")
    make_identity(nc, identity)
    identity16 = consts.tile([P, P], F16, tag="identity16")
    make_identity(nc, identity16)

    def sel16(c, k):
        col = identity16[:k, c:c + 1]
        return bass.AP(col.tensor, col.offset, [list(col.ap[0]), [0, P]])

    xTs = [xTf[:, s] for s in range(NS)]
    xT16s = [xT16f[:, s] for s in range(NS)]

    # adjacency tiles: one (P, NB, M) fp16 tile per stream; masked-max ops
    # broadcast it across the G channel-group dim with 0-stride APs.
    a16s = [consts.tile([P, NB, M], F16, tag=f"a16_{s}", name=f"a16_{s}")
            for s in range(NS)]

    def adjacency(s):
        """diff-based (not Gram) to dodge cancellation near R.
        Stream 0 runs fully on the DVE (startup path, scalar queue must
        stay clear for the first broadcast copies); stream 1 computes the
        squared diffs on the SCALAR engine as Square(bcx + (-x_i))
        (host packs stream 1's x columns negated), overlapping stream 0's
        masked-max DVE work and leaving only adds + compare on the DVE."""
        bcx = []
        for d in range(3):
            ps = psum.tile([P, M], F32, tag="aux", name=f"bcx_ps{s}_{d}")
            col = identity[:3, d:d + 1]
            sel3 = bass.AP(col.tensor, col.offset,
                           [list(col.ap[0]), [0, P]])
            nc.tensor.matmul(ps, sel3, xTs[s], start=True, stop=True)
            b = scratch_pool.tile([P, M], F32, tag="bcx", name=f"bcx{s}_{d}")
            nc.scalar.copy(out=b, in_=ps)
            bcx.append(b)
        for ib in range(NB):
            if s == 0:
                acc = scratch_pool.tile([P, M], F32, tag="adj_acc")
                for d in range(3):
                    dif = scratch_pool.tile([P, M], F32, tag="adj_dif")
                    nc.vector.tensor_scalar(
                        out=dif, in0=bcx[d], scalar1=xs[s][:, ib, d:d + 1],
                        scalar2=None, op0=ALU.subtract,
                    )
                    if d == 0:
                        nc.vector.tensor_mul(acc, dif, dif)
                    else:
                        sq = scratch_pool.tile([P, M], F32, tag="adj_sq")
                        nc.vector.tensor_mul(sq, dif, dif)
                        nc.vector.tensor_add(acc, acc, sq)
            else:
                sqs = []
                for d in range(3):
                    sq_d = scratch_pool.tile([P, M], F32, tag=f"adj_sq{d}",
                                             name=f"sq{s}_{ib}_{d}")
                    nc.scalar.activation(
                        out=sq_d, in_=bcx[d], func=AF.Square,
                        bias=xs[s][:, ib, d:d + 1], scale=1.0,
                    )
                    sqs.append(sq_d)
                a1 = scratch_pool.tile([P, M], F32, tag="adj_acc")
                nc.vector.tensor_add(a1, sqs[0], sqs[1])
                acc = scratch_pool.tile([P, M], F32, tag="adj_acc2")
                nc.vector.tensor_add(acc, a1, sqs[2])
            nc.vector.tensor_scalar(
                out=a16s[s][:, ib, :], in0=acc, scalar1=R, scalar2=None,
                op0=ALU.is_lt,
            )

    states = list(xTs)       # fp32, for residual + output
    states16 = list(xT16s)   # fp16 shadow, rhs of the first edge-MLP layer

    def mlp_layer(rhs, wname, bname, t, ndim, relu=True, out_dtype=F16,
                  nm=""):
        ps = psum.tile([ndim, M], F32, tag="mlp", name=f"ps_{nm}")
        nc.tensor.matmul(ps, wt(wname, t), rhs, start=True, stop=True)
        o = work.tile([ndim, M], out_dtype, tag=f"act_{wname}", name=nm)
        nc.scalar.activation(
            out=o, in_=ps, func=AF.Relu if relu else AF.Identity,
            bias=wt(bname, t), scale=1.0,
        )
        return o

    aggblks = {}

    def compute_mm(s, t, pre_dve_hook=None):
        """edge MLP + broadcast + masked max (the DVE phase). All
        broadcast groups are issued before any DVE op so the tensor
        queue never stalls between channel groups."""
        h1T = mlp_layer(states16[s], "fW1s", "fb1", t, 64, nm=f"h1_{s}_{t}")
        h2T = mlp_layer(h1T, "fW2", "fb2", t, C, nm=f"h2_{s}_{t}")
        ET = mlp_layer(h2T, "fW3c", "fb3c", t, CH, relu=False,
                       nm=f"ET_{s}_{t}")

        a16 = a16s[s]
        adj_bc = bass.AP(a16.tensor, a16.offset,
                         [list(a16.ap[0]), list(a16.ap[1]), [0, G],
                          list(a16.ap[2])])
        H = CH // 2
        aggblk_h = [work.tile([P, NB, H], F16, tag=f"aggblk{h}",
                              name=f"aggblk{h}_{s}_{t}") for h in range(2)]
        ebcgs = []
        for cg in range(NG):
            ebcg = ebc_pool.tile([P, G, M], F16, tag="ebcg")
            for cc in range(G):
                ps = psum_bc.tile([P, M], F32, tag="ebc",
                                  name=f"ebc{t}_{s}_{cg}_{cc}")
                nc.tensor.matmul(ps, sel16(cg * G + cc, CH), ET,
                                 start=True, stop=True)
                nc.scalar.copy(out=ebcg[:, cc, :], in_=ps)
            ebcgs.append(ebcg)
        if pre_dve_hook is not None:
            pre_dve_hook()
        for cg in range(NG):
            ebcg = ebcgs[cg]
            ebc_bc = bass.AP(ebcg.tensor, ebcg.offset,
                             [list(ebcg.ap[0]), [0, NB], list(ebcg.ap[1]),
                              list(ebcg.ap[2])])
            mg = mg_pool.tile([P, NB, G, M], F16, tag="mgrp")
            mg2 = mg_pool.tile([P, NB, G, M // 2], F16, tag="mgrp2")
            nc.vector.tensor_tensor(out=mg, in0=adj_bc, in1=ebc_bc,
                                    op=ALU.mult)
            nc.vector.tensor_tensor(
                out=mg2, in0=mg[:, :, :, :192], in1=mg[:, :, :, 192:],
                op=ALU.max)
            nc.vector.tensor_tensor(
                out=mg[:, :, :, :96], in0=mg2[:, :, :, :96],
                in1=mg2[:, :, :, 96:], op=ALU.max)
            nc.vector.tensor_tensor(
                out=mg2[:, :, :, :48], in0=mg[:, :, :, :48],
                in1=mg[:, :, :, 48:96], op=ALU.max)
            nc.vector.tensor_tensor(
                out=mg[:, :, :, :24], in0=mg2[:, :, :, :24],
                in1=mg2[:, :, :, 24:48], op=ALU.max)
            nc.vector.tensor_tensor(
                out=mg2[:, :, :, :12], in0=mg[:, :, :, :12],
                in1=mg[:, :, :, 12:24], op=ALU.max)
            nc.vector.tensor_tensor(
                out=mg[:, :, :, :6], in0=mg2[:, :, :, :6],
                in1=mg2[:, :, :, 6:12], op=ALU.max)
            h, hc = divmod(cg, NG // 2)
            nc.vector.tensor_reduce(
                out=aggblk_h[h][:, :, hc * G:(hc + 1) * G],
                in_=mg[:, :, :, :6], axis=AX.X, op=ALU.max,
            )
        aggblks[s] = aggblk_h

    def compute_fin(s, t):
        """transpose agg to (CH, M), store, launch AllGather. Per-half
        transposes so the first half starts under the second half's
        vector work."""
        aggblk_h = aggblks[s]
        H = CH // 2
        for h in range(2):
            aggTh = work.tile([H, M], F16, tag=f"aggT{h}",
                              name=f"aggT{h}_{s}_{t}")
            for ib in range(NB):
                ps = psum.tile([H, P], F16, tag="aux",
                               name=f"tr_agg{t}_{s}_{h}_{ib}")
                nc.tensor.transpose(ps, aggblk_h[h][:, ib, :], identity16)
                nc.scalar.copy(out=aggTh[:, ib * P:(ib + 1) * P], in_=ps)
            nc.sync.dma_start(out=agg_out[t][s][h * H:(h + 1) * H, :],
                              in_=aggTh)
        nc.gpsimd.collective_compute(
            "AllGather", ALU.bypass, replica_groups=REPLICA_GROUPS,
            ins=[agg_out[t][s][:]], outs=[agg_full[t][s][:]],
        )

    def _g_tail(s, t, ps_g1):
        g1T = work.tile([64, M], F16, tag="g1T", name=f"g1T_{s}_{t}")
        nc.scalar.activation(out=g1T, in_=ps_g1, func=AF.Relu,
                             bias=wt("gb1", t), scale=1.0)
        g2T = mlp_layer(g1T, "gW2", "gb2", t, 32, nm=f"g2_{s}_{t}")
        gdT = mlp_layer(g2T, "gW3", "gb3", t, 3, out_dtype=F32,
                        nm=f"g3_{s}_{t}")
        newT = work.tile([3, M], F32, tag=f"stateT{s}", name=f"stateT{s}_{t}")
        if t == T - 1:
            # tail: the vector engine is idle here and an add is cheapest
            nc.vector.tensor_add(newT, gdT, states[s])
            states[s] = newT
            return
        # state residual via identity-matmul accumulation (keeps the add
        # off the vector engine mid-pipeline; relu must precede the add)
        ps_n = psum.tile([3, M], F32, tag="mlp", name=f"ps_n_{s}_{t}")
        nc.tensor.matmul(ps_n, identity[:3, :3], gdT, start=True, stop=False)
        nc.tensor.matmul(ps_n, identity[:3, :3], states[s], start=False,
                         stop=True)
        nc.scalar.copy(out=newT, in_=ps_n)
        states[s] = newT
        newT16 = work.tile([3, M], F16, tag=f"stateT16_{s}",
                           name=f"stateT16_{s}_{t}")
        nc.scalar.copy(out=newT16, in_=ps_n)
        states16[s] = newT16

    def g_phase(s, t):
        """gather in, MLP_g; the +state residual is accumulated into a
        PSUM by identity matmuls so no DVE op is involved."""
        aggF = work.tile([C, M], F16, tag=f"aggF{s}",
                         name=f"aggF{t}_{s}")
        nc.sync.dma_start(
            out=aggF,
            in_=agg_full[t][s][:].rearrange("r c m -> (r c) m"))
        ps_g1 = psum_g.tile([64, M], F32, tag=f"psg1_{s}", name=f"psg1_{s}{t}")
        nc.tensor.matmul(ps_g1, wt("gW1", t), aggF, start=True, stop=True)
        _g_tail(s, t, ps_g1)

    def out_phase(s):
        nc.sync.dma_start(out=out_ext[s], in_=states[s])

    # ---- software-pipelined schedule ----
    # A_mm = compute_mm (DVE-heavy), A_fin = compute_fin, B = g_phase.
    # Steady state: B(s,t) + A_mm(s,t+1)'s tensor/scalar head run under
    # the OTHER stream's A_mm DVE work.
    adjacency(0)
    compute_mm(0, 0)
    adjacency(1)
    compute_fin(0, 0)
    compute_mm(1, 0)
    for t in range(T):
        g_phase(0, t)
        if t == T - 1:
            out_phase(0)
        else:
            compute_mm(0, t + 1)
        compute_fin(1, t)
        g_phase(1, t)
        if t == T - 1:
            out_phase(1)
        if t < T - 1:
            if t + 1 == T - 1:
                compute_mm(1, t + 1,
                           pre_dve_hook=lambda: compute_fin(0, T - 1))
            else:
                compute_mm(1, t + 1)
                compute_fin(0, t + 1)


_NC_CACHE = None


def _build_nc():
    global _NC_CACHE
    if _NC_CACHE is None:
        nc = bacc.Bacc(
            "TRN2", target_bir_lowering=False, debug=False,
            num_devices=N_CORES,
        )
        with ExitStack() as ctx:
            tc = ctx.enter_context(tile.TileContext(nc))
            build(ctx, tc)
        nc.compile()
        _NC_CACHE = nc
    return _NC_CACHE


def _pack_blobs(inputs, r):
    """Pack per-core weight blobs. r = channel-slice index (0..3)."""
    sl = slice(CH * r, CH * r + CH)
    perm = np.arange(C).reshape(4, 2, CH // 2)
    w16 = {
        "fW1s": inputs["fW1"][:, 3:6, :],
        "fW2": inputs["fW2"],
        "fW3c": inputs["fW3"][:, :, sl],
        "gW1": inputs["gW1"],
        "gW1h0": inputs["gW1"][:, perm[:, 0].ravel(), :],
        "gW1h1": inputs["gW1"][:, perm[:, 1].ravel(), :],
        "gW2": inputs["gW2"],
        "gW3": inputs["gW3"],
    }
    b32 = {
        "fb1": inputs["fb1"], "fb2": inputs["fb2"],
        "fb3c": inputs["fb3"][:, sl], "gb1": inputs["gb1"],
        "gb2": inputs["gb2"], "gb3": inputs["gb3"],
    }
    wb16 = np.zeros((P, T * _W16_COLS), np.float16)
    for t in range(T):
        for name, rows, cols in _W16:
            o = _w16_off(name, t)
            wb16[:rows, o:o + cols] = w16[name][t].astype(np.float16)
    wb32 = np.zeros((P, T * len(_B32)), np.float32)
    for t in range(T):
        for name, rows in _B32:
            wb32[:rows, _b32_off(name, t)] = b32[name][t]
    return wb16, wb32


def _in_maps(inputs):
    maps = []
    for k in range(N_CORES):
        q, r = k // 4, k % 4
        wb16, wb32 = _pack_blobs(inputs, r)
        xs = inputs["x"][2 * q:2 * q + 2]            # (NS, M, 3)
        xp = np.ascontiguousarray(
            xs.reshape(NS, NB, P, 3).transpose(2, 0, 1, 3))
        xT = np.ascontiguousarray(xs.transpose(2, 0, 1))   # (3, NS, M)
        xp[:, 1] = -xp[:, 1]
        maps.append({
            "x": np.ascontiguousarray(xp),
            "xT": xT,
            "xT16": xT.astype(np.float16),
            "wb16": wb16,
            "wb32": wb32,
        })
    return maps


_WARMED = [False]


def kernel(trace=False, **inputs):
    _register_ntff_hook()
    nc = _build_nc()
    inputs = {k: np.asarray(v, np.float32) for k, v in inputs.items()}
    maps = _in_maps(inputs)
    if not _WARMED[0]:
        # one throwaway execution: the first NEFF run pays a one-time
        # multi-core startup skew (~70us) at the first collective
        run_bass_kernel_spmd(nc, maps, list(range(N_CORES)), trace=False)
        _WARMED[0] = True
    res = run_bass_kernel_spmd(
        nc, maps, list(range(N_CORES)), trace=trace,
    )
    out = np.stack([res.results[4 * (f // 2)]["out"][f % 2].T
                    for f in range(N_FRAMES)])
    if trace:
        kernel.last_results = res
    return out.astype(np.float32)



# revision 6
# speedup vs baseline: 2.7947x; 2.7947x over previous
"""PointGNN Trainium2 kernel (nn_PointGNN_11931419149118) — LSE edition.

Algebraic collapse (same as before): the edge-MLP input is
concat(zeros(3), adj ? state[j] : 0), so the edge feature depends only
on the source j:  agg[i, c] = max(0, max_{j in N(i)} E'[j, c])  with
E' = pre-relu edge MLP of the states.

NEW: the masked max over neighbors is computed on the TENSOR engine as
a log-sum-exp matmul instead of a ~245us dense mask+max-tree on the
vector engine.  Because adjacency is symmetric, adj blocks serve as
matmul lhsT directly:

    S_lo[i,c] = sum_j adj[i,j] * exp(B  * (E'[j,c] - m_c))     B = 80
    S_hi[i,c] = sum_j adj[i,j] * exp(2B * (E'[j,c] - m_c))
    est = m_c + (ln S_hi - ln S_lo)/B     (ratio estimator, err ~ -0.2/B)
    fallback (ln S_lo underflow zone, ~2.6% of cells, picked by the
    L_lo >= -35 predicate so hw Ln(0) behavior is irrelevant):
    est = m_c + (ln S_lo + 0.2)/B
    agg = relu(est)  == the reference's masked max to ~4e-2 abs, giving
    ~5e-3 final rel err (validated bit-faithfully in numpy vs the jax
    reference; gate is 2e-2).

m_c = max_j E'[j,c] keeps exp in range; exp args are clamped to >= -87
(bf16/fp32 normal range) before the act-table sees them; F tiles are
bf16 for exponent range.  The exp is applied for free on the scalar
engine during the PSUM->SBUF copy after the PE transposes of the
pre-scaled E tiles, and relu(est/B + m_c) lands on the transposed-back
agg tiles the same way.

With the DVE freed, each core computes its TWO frames at FULL channel
width: no AllGather, no collectives at all.  Cores 4q..4q+3 all own
frames {2q, 2q+1}; the host reads cores 0 and 4.
"""

import sys
import types

sys.path.insert(0, "/opt/trn_rl_repo")

import numpy as np
from contextlib import ExitStack

import concourse.bass as bass
import concourse.mybir as mybir
import concourse.tile as tile
from concourse import bacc
from concourse.bass_utils import run_bass_kernel_spmd
from concourse.masks import make_identity

F32 = mybir.dt.float32
F16 = mybir.dt.float16
BF16 = mybir.dt.bfloat16
AF = mybir.ActivationFunctionType
ALU = mybir.AluOpType
AX = mybir.AxisListType

N_FRAMES = 4
M = 384          # points per frame
P = 128          # partitions
NB = M // P      # 3 blocks of destination points
T = 3            # timesteps
C = 128          # edge channels
NS = 2           # frames per core
R = 0.05         # squared-distance threshold
N_CORES = 8

BETA = 76.0      # LSE moment scale (moments at BETA and 2*BETA)
GAM = -0.4       # fallback bias correction
CLAMP = -87.0    # exp arg clamp (bf16/fp32 normal range)
# hw Ln act table is only accurate for inputs in [e^-43.5, e^35] (saturates
# at ~-45.8 below, drifts above) so both sums are pre-scaled inside the Ln
# activation to land in that window; the constants cancel downstream.
K_LO = 28.5      # ln(scale) for S_lo:  L_lo = ln(S_lo) + K_LO
K_HI = 26.0      # ln(scale) for S_hi:  L_hi = ln(S_hi) + K_HI
PRED_THR = -30.0 + K_LO   # on L_lo: ratio branch iff lambda >= -30
KILL_THR = -40.0          # on L_lo: below this S_lo underflowed -> agg 0
KILL_VAL = -50000.0

# packed fp16 weight blob layout: per t, (rows, cols) per weight
_W16 = [("fW1s", 3, 64), ("fW2", 64, C), ("fW3", C, C),
        ("gW1", C, 64), ("gW2", 64, 32), ("gW3", 32, 3)]
_W16_COLS = sum(c for _, _, c in _W16)           # per timestep
_B32 = [("fb1", 64), ("fb2", C), ("fb3", C), ("gb1", 64),
        ("gb2", 32), ("gb3", 3)]


def _w16_off(name, t):
    off = t * _W16_COLS
    for n, _, c in _W16:
        if n == name:
            return off
        off += c
    raise KeyError(name)


def _b32_off(name, t):
    off = t * len(_B32)
    for i, (n, _) in enumerate(_B32):
        if n == name:
            return off + i
    raise KeyError(name)


def _register_ntff_hook():
    """Register the axon NTFF profile hook the image's antenv lacks."""
    try:
        import antenv
        if "antenv.axon_hooks" in sys.modules:
            return
        mod = types.ModuleType("antenv.axon_hooks")
        _hook = [None]
        mod.set_axon_ntff_profile_hook = lambda h: _hook.__setitem__(0, h)
        mod.get_axon_ntff_profile_hook = lambda: _hook[0]
        sys.modules["antenv.axon_hooks"] = mod
        antenv.axon_hooks = mod
        from trn_agent_boot.trn_boot import _ntff_profile_via_ctypes
        mod.set_axon_ntff_profile_hook(
            _ntff_profile_via_ctypes("/opt/axon/libaxon_pjrt.so")
        )
    except Exception:
        pass


def build(ctx, tc):
    nc = tc.nc

    x_in = nc.declare_dram_parameter("x", [P, NS, NB, 3], F32,
                                     isOutput=False)
    xT_in = nc.declare_dram_parameter("xT", [3, NS, M], F32, isOutput=False)
    xT16_in = nc.declare_dram_parameter("xT16", [3, NS, M], F16,
                                        isOutput=False)
    wb16_in = nc.declare_dram_parameter("wb16", [P, T * _W16_COLS], F16,
                                        isOutput=False)
    wb32_in = nc.declare_dram_parameter("wb32", [P, T * len(_B32)], F32,
                                        isOutput=False)
    out_ext = nc.declare_dram_parameter("out", [NS, 3, M], F32, isOutput=True)

    consts = ctx.enter_context(tc.tile_pool(name="consts", bufs=1))
    scratch = ctx.enter_context(tc.tile_pool(name="scratch", bufs=3))
    work = ctx.enter_context(tc.tile_pool(name="work", bufs=2))
    psum = ctx.enter_context(
        tc.tile_pool(name="psum", bufs=2, space=bass.MemorySpace.PSUM)
    )
    psum_et = ctx.enter_context(
        tc.tile_pool(name="psum_et", bufs=1, space=bass.MemorySpace.PSUM)
    )
    psum_S = ctx.enter_context(
        tc.tile_pool(name="psum_S", bufs=3, space=bass.MemorySpace.PSUM)
    )
    psum_tr = ctx.enter_context(
        tc.tile_pool(name="psum_tr", bufs=2, space=bass.MemorySpace.PSUM)
    )

    # ---- input DMAs ----
    xall = consts.tile([P, NS, NB, 3], F32, tag="xall", name="xall")
    nc.sync.dma_start(out=xall, in_=x_in[:])
    xs = [xall[:, s] for s in range(NS)]
    xTf = consts.tile([3, NS, M], F32, tag="xTf", name="xTf")
    nc.sync.dma_start(out=xTf, in_=xT_in[:])
    xT16f = consts.tile([3, NS, M], F16, tag="xT16f", name="xT16f")
    nc.sync.dma_start(out=xT16f, in_=xT16_in[:])
    wb16 = consts.tile([P, T * _W16_COLS], F16, tag="wb16", name="wb16")
    nc.sync.dma_start(out=wb16, in_=wb16_in[:])
    wb32 = consts.tile([P, T * len(_B32)], F32, tag="wb32", name="wb32")
    nc.sync.dma_start(out=wb32, in_=wb32_in[:])

    def wt(name, t):
        for n, r, c in _W16:
            if n == name:
                o = _w16_off(name, t)
                return wb16[:r, o:o + c]
        for n, r in _B32:
            if n == name:
                return wb32[:r, _b32_off(name, t):_b32_off(name, t) + 1]
        raise KeyError(name)

    identity = consts.tile([P, P], F32, tag="identity")
    make_identity(nc, identity)
    identity16 = consts.tile([P, P], F16, tag="identity16")
    make_identity(nc, identity16)

    def sel3(d):
        col = identity[:3, d:d + 1]
        return bass.AP(col.tensor, col.offset, [list(col.ap[0]), [0, P]])

    xTs = [xTf[:, s] for s in range(NS)]
    xT16s = [xT16f[:, s] for s in range(NS)]

    # adjacency: a16[p, b, j] = adj[b*128+p, j] in bf16 (0/1).  Symmetric,
    # so a16[:, jb, ib*128:(ib+1)*128] is the lhsT block for (ib, jb).
    a16s = [consts.tile([P, NB, M], BF16, tag=f"a16_{s}", name=f"a16_{s}")
            for s in range(NS)]

    def adjacency(s):
        """d2 = sum_d (x_j[d] - x_i[d])^2 via scalar-engine Square with the
        host-negated x_i as per-partition bias; exact diff-based form (the
        closest pair sits 3.9e-6 from R, Gram form would be unsafe)."""
        bcx = []
        for d in range(3):
            ps = psum_S.tile([P, M], F32, tag="S", name=f"bcx{s}_{d}")
            nc.tensor.matmul(ps, sel3(d), xTs[s], start=True, stop=True)
            bcx.append(ps)
        for ib in range(NB):
            sqs = []
            for d in range(3):
                sq_d = scratch.tile([P, M], F32, tag=f"adj_sq{d}",
                                    name=f"sq{s}_{ib}_{d}")
                nc.scalar.activation(
                    out=sq_d, in_=bcx[d], func=AF.Square,
                    bias=xs[s][:, ib, d:d + 1], scale=1.0,
                )
                sqs.append(sq_d)
            a1 = scratch.tile([P, M], F32, tag="adj_acc")
            nc.vector.tensor_add(a1, sqs[0], sqs[1])
            acc = scratch.tile([P, M], F32, tag="adj_acc2")
            nc.vector.tensor_add(acc, a1, sqs[2])
            nc.vector.tensor_scalar(
                out=a16s[s][:, ib, :], in0=acc, scalar1=R, scalar2=None,
                op0=ALU.is_lt,
            )

    states = list(xTs)       # fp32, residual + output
    states16 = list(xT16s)   # fp16 shadow, rhs of the first edge-MLP layer

    # per-stream tiles that cross phase boundaries
    Fcats = {}
    m_relus = {}
    aggFs = {}

    def edge_phase(s, t):
        """edge MLP -> E' (PSUM), m_c, pre-scaled+clamped exp args, PE
        transposes, and the exp copies producing F = [F_lo | F_hi] bf16."""
        ps_h1 = psum.tile([64, M], F32, tag="mlp", name=f"ps_h1_{s}_{t}")
        nc.tensor.matmul(ps_h1, wt("fW1s", t), states16[s], start=True,
                         stop=True)
        h1 = work.tile([64, M], F16, tag="h1", name=f"h1_{s}_{t}")
        nc.scalar.activation(out=h1, in_=ps_h1, func=AF.Relu,
                             bias=wt("fb1", t), scale=1.0)
        ps_h2 = psum.tile([C, M], F32, tag="mlp", name=f"ps_h2_{s}_{t}")
        nc.tensor.matmul(ps_h2, wt("fW2", t), h1, start=True, stop=True)
        h2 = work.tile([C, M], F16, tag="h2", name=f"h2_{s}_{t}")
        nc.scalar.activation(out=h2, in_=ps_h2, func=AF.Relu,
                             bias=wt("fb2", t), scale=1.0)
        ps_et = psum_et.tile([C, M], F32, tag="et", name=f"ps_et_{s}_{t}")
        nc.tensor.matmul(ps_et, wt("fW3", t), h2, start=True, stop=True)

        # m' = max_j (E' - fb3) per channel; fb3 cancels in B*(E'-m_c) so
        # the exp args never need fb3; only the final relu bias does.
        mprime = work.tile([C, 1], F32, tag="mprime", name=f"mp_{s}_{t}")
        nc.vector.tensor_reduce(out=mprime, in_=ps_et, axis=AX.X, op=ALU.max)
        negBm = work.tile([C, 1], F32, tag="negBm", name=f"negBm_{s}_{t}")
        nc.vector.tensor_scalar(out=negBm, in0=mprime, scalar1=-BETA,
                                scalar2=None, op0=ALU.mult)
        m_relu = work.tile([C, 1], F32, tag=f"m_relu{s}", name=f"mr_{s}_{t}")
        nc.vector.tensor_tensor(out=m_relu, in0=mprime, in1=wt("fb3", t),
                                op=ALU.add)
        m_relus[s] = m_relu

        etp1 = work.tile([C, M], F16, tag="etp1", name=f"etp1_{s}_{t}")
        nc.scalar.activation(out=etp1, in_=ps_et, func=AF.Identity,
                             bias=negBm, scale=BETA)
        etp1c = work.tile([C, M], F16, tag="etp1c", name=f"etp1c_{s}_{t}")
        nc.vector.tensor_scalar(out=etp1c, in0=etp1, scalar1=CLAMP,
                                scalar2=None, op0=ALU.max)
        etp2 = work.tile([C, M], F16, tag="etp2", name=f"etp2_{s}_{t}")
        nc.vector.tensor_scalar(out=etp2, in0=etp1, scalar1=2.0,
                                scalar2=CLAMP, op0=ALU.mult, op1=ALU.max)

        Fcat = [work.tile([P, 2 * C], BF16, tag=f"Fcat{jb}",
                          name=f"F{jb}_{s}_{t}") for jb in range(NB)]
        for jb in range(NB):
            lo = P * jb
            ps_t1 = psum_tr.tile([P, P], F16, tag="tr",
                                 name=f"tr1_{s}_{t}_{jb}")
            nc.tensor.transpose(ps_t1, etp1c[:, lo:lo + P], identity16)
            nc.scalar.activation(out=Fcat[jb][:, :C], in_=ps_t1, func=AF.Exp)
            ps_t2 = psum_tr.tile([P, P], F16, tag="tr",
                                 name=f"tr2_{s}_{t}_{jb}")
            nc.tensor.transpose(ps_t2, etp2[:, lo:lo + P], identity16)
            nc.scalar.activation(out=Fcat[jb][:, C:], in_=ps_t2, func=AF.Exp)
        Fcats[s] = Fcat

    def smax_phase(s, t):
        """S = adj @ F (PE), ln (scalar), combine (DVE), transpose back,
        relu-affine copy -> aggF [C, M]."""
        Fcat = Fcats[s]
        Lcat = work.tile([P, NB, 2 * C], F16, tag="Lcat", name=f"L_{s}_{t}")
        for ib in range(NB):
            ps_S = psum_S.tile([P, 2 * C], F32, tag="S",
                               name=f"S_{s}_{t}_{ib}")
            for jb in range(NB):
                nc.tensor.matmul(
                    ps_S, a16s[s][:, jb, P * ib:P * ib + P], Fcat[jb],
                    start=(jb == 0), stop=(jb == NB - 1),
                )
            nc.scalar.activation(out=Lcat[:, ib, :C], in_=ps_S[:, :C],
                                 func=AF.Ln, scale=float(np.exp(K_LO)))
            nc.scalar.activation(out=Lcat[:, ib, C:], in_=ps_S[:, C:],
                                 func=AF.Ln, scale=float(np.exp(K_HI)))
        Llo = Lcat[:, :, :C]
        Lhi = Lcat[:, :, C:]
        # q = KILL_VAL where S_lo fully underflowed (true agg <= 0 there)
        q3 = work.tile([P, NB, C], F16, tag="q3", name=f"q3_{s}_{t}")
        nc.vector.tensor_scalar(out=q3, in0=Llo, scalar1=KILL_THR,
                                scalar2=KILL_VAL, op0=ALU.is_lt,
                                op1=ALU.mult)
        dfb = work.tile([P, NB, C], F16, tag="dfb", name=f"dfb_{s}_{t}")
        nc.vector.tensor_scalar(out=dfb, in0=Llo,
                                scalar1=GAM + (K_HI - 2.0 * K_LO),
                                scalar2=None, op0=ALU.add)
        D3 = work.tile([P, NB, C], F16, tag="D3", name=f"D3_{s}_{t}")
        nc.vector.tensor_tensor(out=D3, in0=dfb, in1=q3, op=ALU.add)
        pred = work.tile([P, NB, C], F16, tag="pred", name=f"pr_{s}_{t}")
        nc.vector.tensor_scalar(out=pred, in0=Llo, scalar1=PRED_THR,
                                scalar2=None, op0=ALU.is_ge)
        dsub = work.tile([P, NB, C], F16, tag="dsub", name=f"ds_{s}_{t}")
        nc.vector.tensor_tensor(out=dsub, in0=Lhi, in1=Llo, op=ALU.subtract)
        nc.vector.copy_predicated(D3, pred.bitcast(mybir.dt.uint16), dsub)

        aggF = work.tile([C, M], F16, tag=f"aggF{s}", name=f"agg_{s}_{t}")
        for ib in range(NB):
            ps_d = psum_tr.tile([P, P], F16, tag="tr",
                                name=f"dtr_{s}_{t}_{ib}")
            nc.tensor.transpose(ps_d, D3[:, ib, :], identity16)
            nc.scalar.activation(out=aggF[:, P * ib:P * ib + P], in_=ps_d,
                                 func=AF.Relu, bias=m_relus[s],
                                 scale=1.0 / BETA)
        aggFs[s] = aggF

    def g_phase(s, t):
        ps_g1 = psum.tile([64, M], F32, tag="mlp", name=f"ps_g1_{s}_{t}")
        nc.tensor.matmul(ps_g1, wt("gW1", t), aggFs[s], start=True, stop=True)
        g1 = work.tile([64, M], F16, tag="g1", name=f"g1_{s}_{t}")
        nc.scalar.activation(out=g1, in_=ps_g1, func=AF.Relu,
                             bias=wt("gb1", t), scale=1.0)
        ps_g2 = psum.tile([32, M], F32, tag="mlp", name=f"ps_g2_{s}_{t}")
        nc.tensor.matmul(ps_g2, wt("gW2", t), g1, start=True, stop=True)
        g2 = work.tile([32, M], F16, tag="g2", name=f"g2_{s}_{t}")
        nc.scalar.activation(out=g2, in_=ps_g2, func=AF.Relu,
                             bias=wt("gb2", t), scale=1.0)
        ps_g3 = psum.tile([3, M], F32, tag="mlp", name=f"ps_g3_{s}_{t}")
        nc.tensor.matmul(ps_g3, wt("gW3", t), g2, start=True, stop=True)
        gd = work.tile([3, M], F32, tag="gd", name=f"gd_{s}_{t}")
        nc.scalar.activation(out=gd, in_=ps_g3, func=AF.Relu,
                             bias=wt("gb3", t), scale=1.0)
        newT = work.tile([3, M], F32, tag=f"stateT{s}", name=f"st_{s}_{t}")
        nc.vector.tensor_add(newT, gd, states[s])
        states[s] = newT
        if t < T - 1:
            new16 = work.tile([3, M], F16, tag=f"stateT16_{s}",
                              name=f"st16_{s}_{t}")
            nc.vector.tensor_copy(out=new16, in_=newT)
            states16[s] = new16

    # ---- schedule ----
    adjacency(0)
    adjacency(1)
    for t in range(T):
        edge_phase(0, t)
        edge_phase(1, t)
        smax_phase(0, t)
        smax_phase(1, t)
        g_phase(0, t)
        g_phase(1, t)
    for s in range(NS):
        nc.sync.dma_start(out=out_ext[s], in_=states[s])


_NC_CACHE = None


def _build_nc():
    global _NC_CACHE
    if _NC_CACHE is None:
        nc = bacc.Bacc(
            "TRN2", target_bir_lowering=False, debug=False,
            num_devices=N_CORES,
        )
        with ExitStack() as ctx:
            tc = ctx.enter_context(tile.TileContext(nc))
            build(ctx, tc)
        nc.compile()
        _NC_CACHE = nc
    return _NC_CACHE


def _pack_blobs(inputs):
    w16 = {
        "fW1s": inputs["fW1"][:, 3:6, :],
        "fW2": inputs["fW2"],
        "fW3": inputs["fW3"],
        "gW1": inputs["gW1"],
        "gW2": inputs["gW2"],
        "gW3": inputs["gW3"],
    }
    b32 = {
        "fb1": inputs["fb1"], "fb2": inputs["fb2"],
        "fb3": inputs["fb3"] - np.float32((K_HI - K_LO) / BETA),
        "gb1": inputs["gb1"], "gb2": inputs["gb2"], "gb3": inputs["gb3"],
    }
    wb16 = np.zeros((P, T * _W16_COLS), np.float16)
    for t in range(T):
        for name, rows, cols in _W16:
            o = _w16_off(name, t)
            wb16[:rows, o:o + cols] = w16[name][t].astype(np.float16)
    wb32 = np.zeros((P, T * len(_B32)), np.float32)
    for t in range(T):
        for name, rows in _B32:
            wb32[:rows, _b32_off(name, t)] = b32[name][t]
    return wb16, wb32


def _in_maps(inputs):
    wb16, wb32 = _pack_blobs(inputs)
    maps = []
    for k in range(N_CORES):
        q = k // 4
        xsl = inputs["x"][2 * q:2 * q + 2]            # (NS, M, 3)
        xp = np.ascontiguousarray(
            xsl.reshape(NS, NB, P, 3).transpose(2, 0, 1, 3))
        xT = np.ascontiguousarray(xsl.transpose(2, 0, 1))   # (3, NS, M)
        maps.append({
            "x": np.ascontiguousarray(-xp),   # negated: Square bias = -x_i
            "xT": xT,
            "xT16": xT.astype(np.float16),
            "wb16": wb16,
            "wb32": wb32,
        })
    return maps


_WARMED = [False]


def kernel(trace=False, **inputs):
    _register_ntff_hook()
    nc = _build_nc()
    inputs = {k: np.asarray(v, np.float32) for k, v in inputs.items()}
    maps = _in_maps(inputs)
    if not _WARMED[0]:
        run_bass_kernel_spmd(nc, maps, list(range(N_CORES)), trace=False)
        _WARMED[0] = True
    res = run_bass_kernel_spmd(
        nc, maps, list(range(N_CORES)), trace=trace,
    )
    out = np.stack([res.results[4 * (f // 2)]["out"][f % 2].T
                    for f in range(N_FRAMES)])
    if trace:
        kernel.last_results = res
    return out.astype(np.float32)
